# revision 21
# baseline (speedup 1.0000x reference)
"""RGBD channel-attention (CAM) module on 8 Trainium2 NeuronCores.

Per batch b (one per core, pure data-parallel):
    q  = x_rgb[b].reshape(C, N)          C=512, N=4096
    qd = x_dep[b].reshape(C, N)
    S  = q @ q.T + qd @ qd.T             (512 x 512, symmetric)
    att = softmax(-S, axis=-1)           (max-shift trick cancels in softmax)
    out = gamma * (att @ q) + x_rgb[b]

On-chip schedule per core:
  phase E: build qT / qdT k-tiles via PE transposes, accumulate the upper
           triangle of S = sum_k qT_k^T @ qT_k (both inputs) into 4 PSUM
           banks; matmuls run as float32r (full PE rate).
  S fix:   copy upper blocks to SBUF, mirror the 6 lower blocks via PE
           transposes (S is symmetric).
  softmax: rowwise m=min(S); unnormalized att = exp(-S + m) on ACT with
           accumulated row sum; normalization deferred to the output scale.
  attT:    16 PE transposes (att is needed d-major as matmul lhsT).
  out:     attT.T @ q over d-tiles (f32r); PSUM scaled by gamma/rowsum
           per-partition, residual-added to an exact f32 copy of x, DMA out.
"""

import sys

if "/opt/trn_rl_repo" not in sys.path:
    sys.path.insert(0, "/opt/trn_rl_repo")

from contextlib import ExitStack

import numpy as np

import concourse.bacc as bacc
import concourse.mybir as mybir
import concourse.tile as tile
from concourse import bass_utils
from concourse.masks import make_identity

P = 128          # partitions
C = 512          # channels
N = 4096         # H*W
CT = C // P      # 4 channel tiles
KT = N // P      # 32 contraction tiles per input
FREE = 512       # matmul moving free dim (fp32 max)
NT = N // FREE   # 8 output column tiles
F32 = mybir.dt.float32
F32R = mybir.dt.float32r  # same bits as f32; PE runs full-rate (vs 4 cyc/row for f32)

_NC_CACHE = None


def _emit(ctx, tc, nc, x, xd, g, o):
    xa = x.ap().rearrange("(t p) n -> t p n", p=P)    # [CT, P, N]
    xda = xd.ap().rearrange("(t p) n -> t p n", p=P)
    oa = o.ap().rearrange("(t p) n -> t p n", p=P)

    const = ctx.enter_context(tc.tile_pool(name="const", bufs=1))
    qpool = ctx.enter_context(tc.tile_pool(name="qpool", bufs=1))
    stream = ctx.enter_context(tc.tile_pool(name="stream", bufs=5))
    small = ctx.enter_context(tc.tile_pool(name="small", bufs=1))
    ostream = ctx.enter_context(tc.tile_pool(name="ostream", bufs=3))

    identity = const.tile([P, P], F32)
    make_identity(nc, identity)
    identity_r = const.tile([P, P], F32R, name="identity_r")
    nc.vector.tensor_copy(out=identity_r[:], in_=identity[:])
    gamma_sb = const.tile([P, 1], F32)
    nc.sync.dma_start(gamma_sb[:], g.ap())

    # resident inputs, f32r-rounded by the DMA (matmul/transpose operands only;
    # the exact-x residual is re-streamed in the output phase)
    q_nat = qpool.tile([P, CT, N], F32, name="q_nat", tag="q_nat")
    qd_nat = qpool.tile([P, CT, N], F32R, name="qd_nat", tag="qd_nat")
    # column-block-major load order so the first qT tiles are ready early
    def load_input(dst, dram, cast):
        d = dram.bitcast(F32R) if cast else dram
        cols = [0, 256, 512, 1024, 2048, 3072, 4096]
        i = 0
        for h in range(len(cols) - 1):
            for t in range(CT):
                sl = slice(cols[h], cols[h + 1])
                eng = nc.sync if i % 2 == 0 else nc.scalar
                eng.dma_start(dst[:, t, sl], d[t, :, sl])
                i += 1

    load_input(q_nat, xa, cast=False)
    load_input(qd_nat, xda, cast=True)

    s_sb = qpool.tile([P, CT, FREE], F32, name="s_sb", tag="s_sb")

    with ExitStack() as ectx:
        spsum = ectx.enter_context(tc.tile_pool(name="spsum", bufs=1, space="PSUM"))
        tpsum = ectx.enter_context(tc.tile_pool(name="tpsum", bufs=2, space="PSUM"))

        # S upper-triangle accumulators: one PSUM bank per 128-row tile
        s_ps = [
            spsum.tile([P, FREE], F32, name=f"s{m}", tag=f"s{m}") for m in range(CT)
        ]

        # --- phase E ---
        steps = [(q_nat, k) for k in range(KT)] + [(qd_nat, k) for k in range(KT)]
        n_steps = len(steps)
        pend = {}

        def emit_transpose(i):
            src, k = steps[i]
            is_r = src.dtype == F32R
            tp = tpsum.tile([P, FREE], F32R if is_r else F32, tag="tp", bufs=3)
            for t in range(CT):
                nc.tensor.transpose(
                    tp[:, t * P : (t + 1) * P],
                    src[:, t, k * P : (k + 1) * P],
                    identity_r if is_r else identity,
                )
            qt = stream.tile([P, FREE], F32R, tag="qt")
            if i % 2 == 0:
                nc.vector.tensor_copy(out=qt[:], in_=tp[:])
            else:
                nc.scalar.copy(qt[:], tp[:])
            pend[i] = qt

        def emit_matmuls(i):
            qt = pend.pop(i)
            for m in range(CT):
                lo = m * P
                nc.tensor.matmul(
                    s_ps[m][:, lo:],
                    qt[:, lo : lo + P],
                    qt[:, lo:],
                    start=(i == 0),
                    stop=(i == n_steps - 1),
                )

        # skew by one step: PE transposes of step i+1 overlap the copy of i
        emit_transpose(0)
        for i in range(1, n_steps):
            emit_transpose(i)
            emit_matmuls(i - 1)
        emit_matmuls(n_steps - 1)

        # --- S: upper blocks from PSUM, mirror lower blocks (S symmetric) ---
        for m in range(CT):
            if m % 2 == 0:
                nc.vector.tensor_copy(out=s_sb[:, m, m * P :], in_=s_ps[m][:, m * P :])
            else:
                nc.scalar.copy(s_sb[:, m, m * P :], s_ps[m][:, m * P :])
        for i in range(CT):
            for j in range(i):
                tp = tpsum.tile([P, P], F32, tag="tps", bufs=1)
                nc.tensor.transpose(
                    tp[:], s_sb[:, j, i * P : (i + 1) * P], identity
                )
                nc.vector.tensor_copy(out=s_sb[:, i, j * P : (j + 1) * P], in_=tp[:])

        # --- softmax over rows of -S (stabilizer: rowmax(-S) = -rowmin(S)) ---
        att = qpool.tile([P, CT, FREE], F32, name="att", tag="att")
        rg = []
        for m in range(CT):
            mn = small.tile([P, 1], F32, tag=f"mn{m}")
            nc.vector.tensor_reduce(
                out=mn[:],
                in_=s_sb[:, m, :],
                axis=mybir.AxisListType.X,
                op=mybir.AluOpType.min,
            )
            z = small.tile([P, 1], F32, tag=f"z{m}")
            nc.scalar.activation(
                att[:, m, :],
                s_sb[:, m, :],
                mybir.ActivationFunctionType.Exp,
                bias=mn[:],
                scale=-1.0,
                accum_out=z[:],
            )
            r = small.tile([P, 1], F32, tag=f"r{m}")
            nc.vector.reciprocal(r[:], z[:])
            rgm = small.tile([P, 1], F32, tag=f"rg{m}")
            nc.vector.tensor_mul(out=rgm[:], in0=r[:], in1=gamma_sb[:])
            rg.append(rgm)

        # --- attT (d-major unnormalized att) via 16 PE transposes ---
        attT = qpool.tile([P, CT, FREE], F32R, name="attT", tag="attT")
        for mj in range(CT):
            tp = tpsum.tile([P, FREE], F32, tag="tp", bufs=3)
            for mi in range(CT):
                nc.tensor.transpose(
                    tp[:, mi * P : (mi + 1) * P],
                    att[:, mi, mj * P : (mj + 1) * P],
                    identity,
                )
            nc.vector.tensor_copy(out=attT[:, mj, :], in_=tp[:])

    # --- out = rg * (attT.T @ q) + x, streamed over 8 column tiles ---
    opsum = ctx.enter_context(tc.tile_pool(name="opsum", bufs=4, space="PSUM"))

    def emit_qr(nt):
        nsl = slice(nt * FREE, (nt + 1) * FREE)
        tiles = []
        for kk in range(CT):
            qr = stream.tile([P, FREE], F32R, tag="qr", bufs=8)
            if kk % 2 == 0:
                nc.vector.tensor_copy(out=qr[:], in_=q_nat[:, kk, nsl])
            else:
                nc.scalar.copy(qr[:], q_nat[:, kk, nsl])
            tiles.append(qr)
        return tiles

    q_r = {0: emit_qr(0)}
    for nt in range(NT):
        nsl = slice(nt * FREE, (nt + 1) * FREE)
        tiles = q_r.pop(nt)
        for m in range(CT):
            op_ = opsum.tile([P, FREE], F32, tag="op")
            for kk in range(CT):
                nc.tensor.matmul(
                    op_[:],
                    attT[:, kk, m * P : (m + 1) * P],
                    tiles[kk][:],
                    start=(kk == 0),
                    stop=(kk == CT - 1),
                )
            if m == 0 and nt + 1 < NT:
                q_r[nt + 1] = emit_qr(nt + 1)
            t_sb = ostream.tile([P, FREE], F32, tag="t")
            o_sb = ostream.tile([P, FREE], F32, tag="o")
            if nt == NT - 1:
                # fine-grained tail: quarter-width chunks, engines rotated, so
                # the post-matmul epilogue chain pipelines instead of serializing
                for qtr in range(4):
                    qs = slice(qtr * 128, (qtr + 1) * 128)
                    if qtr % 2 == 0:
                        nc.scalar.mul(t_sb[:, qs], op_[:, qs], rg[m][:])
                        nc.vector.tensor_add(
                            out=o_sb[:, qs], in0=t_sb[:, qs],
                            in1=q_nat[:, m, nsl][:, qs],
                        )
                    else:
                        nc.vector.tensor_scalar_mul(t_sb[:, qs], op_[:, qs], rg[m][:])
                        nc.gpsimd.tensor_add(
                            out=o_sb[:, qs], in0=t_sb[:, qs],
                            in1=q_nat[:, m, nsl][:, qs],
                        )
                    nc.sync.dma_start(oa[m, :, nsl][:, qs], o_sb[:, qs])
            else:
                if m < 2:
                    nc.scalar.mul(t_sb[:], op_[:], rg[m][:])
                else:
                    nc.vector.tensor_scalar_mul(t_sb[:], op_[:], rg[m][:])
                if m % 2 == 1 and nt < NT - 2:
                    nc.gpsimd.tensor_add(out=o_sb[:], in0=t_sb[:], in1=q_nat[:, m, nsl])
                else:
                    nc.vector.tensor_add(out=o_sb[:], in0=t_sb[:], in1=q_nat[:, m, nsl])
                nc.sync.dma_start(oa[m, :, nsl], o_sb[:])


def _build_program():
    global _NC_CACHE
    if _NC_CACHE is not None:
        return _NC_CACHE
    nc = bacc.Bacc("TRN2", target_bir_lowering=False, debug=False)
    x = nc.dram_tensor("x", [C, N], F32, kind="ExternalInput")
    xd = nc.dram_tensor("xd", [C, N], F32, kind="ExternalInput")
    g = nc.dram_tensor("g", [P, 1], F32, kind="ExternalInput")
    o = nc.dram_tensor("o", [C, N], F32, kind="ExternalOutput")

    with tile.TileContext(nc) as tc, ExitStack() as ctx:
        _emit(ctx, tc, nc, x, xd, g, o)
    nc.compile()
    _NC_CACHE = nc
    return nc


def kernel(x_rgb: np.ndarray, x_dep: np.ndarray, gamma: np.ndarray) -> np.ndarray:
    B, Cc, H, W = x_rgb.shape
    assert (B, Cc, H * W) == (8, C, N), (B, Cc, H, W)
    nc = _build_program()
    g128 = np.ascontiguousarray(
        np.broadcast_to(np.float32(gamma).reshape(1, 1), (P, 1)), dtype=np.float32
    )
    in_maps = [
        {
            "x": np.ascontiguousarray(x_rgb[b].reshape(C, N), dtype=np.float32),
            "xd": np.ascontiguousarray(x_dep[b].reshape(C, N), dtype=np.float32),
            "g": g128,
        }
        for b in range(B)
    ]
    res = bass_utils.run_bass_kernel_spmd(nc, in_maps, core_ids=list(range(B)))
    out = np.stack([res.results[b]["o"].reshape(Cc, H, W) for b in range(B)])
    return out.astype(np.float32)


# revision 22
# speedup vs baseline: 1.1964x; 1.1964x over previous
"""RGBD channel-attention (CAM) module on 8 Trainium2 NeuronCores.

Per batch b (one per core, pure data-parallel):
    q  = x_rgb[b].reshape(C, N)          C=512, N=4096
    qd = x_dep[b].reshape(C, N)
    S  = q @ q.T + qd @ qd.T             (512 x 512, symmetric)
    att = softmax(-S, axis=-1)           (max-shift trick cancels in softmax)
    out = gamma * (att @ q) + x_rgb[b]

On-chip schedule per core:
  phase E: build qT / qdT k-tiles via PE transposes, accumulate the upper
           triangle of S = sum_k qT_k^T @ qT_k (both inputs) into 4 PSUM
           banks; matmuls run as float32r (full PE rate).
  S fix:   copy upper blocks to SBUF, mirror the 6 lower blocks via PE
           transposes (S is symmetric).
  softmax: rowwise m=min(S); unnormalized att = exp(-S + m) on ACT with
           accumulated row sum; normalization deferred to the output scale.
  attT:    16 PE transposes (att is needed d-major as matmul lhsT).
  out:     attT.T @ q over d-tiles (f32r); PSUM scaled by gamma/rowsum
           per-partition, residual-added to an exact f32 copy of x, DMA out.
"""

import sys

if "/opt/trn_rl_repo" not in sys.path:
    sys.path.insert(0, "/opt/trn_rl_repo")

from contextlib import ExitStack

import numpy as np

import concourse.bacc as bacc
import concourse.mybir as mybir
import concourse.tile as tile
from concourse import bass_utils
from concourse.masks import make_identity

P = 128          # partitions
C = 512          # channels
N = 4096         # H*W
CT = C // P      # 4 channel tiles
KT = N // P      # 32 contraction tiles per input
FREE = 512       # matmul moving free dim (fp32 max)
NT = N // FREE   # 8 output column tiles
F32 = mybir.dt.float32
F32R = mybir.dt.float32r  # same bits as f32; PE runs full-rate (vs 4 cyc/row for f32)

_NC_CACHE = None


def _emit(ctx, tc, nc, x, xd, g, o):
    xa = x.ap().rearrange("(t p) n -> t p n", p=P)    # [CT, P, N]
    xda = xd.ap().rearrange("(t p) n -> t p n", p=P)
    oa = o.ap().rearrange("(t p) n -> t p n", p=P)

    const = ctx.enter_context(tc.tile_pool(name="const", bufs=1))
    qpool = ctx.enter_context(tc.tile_pool(name="qpool", bufs=1))
    stream = ctx.enter_context(tc.tile_pool(name="stream", bufs=5))
    small = ctx.enter_context(tc.tile_pool(name="small", bufs=1))
    ostream = ctx.enter_context(tc.tile_pool(name="ostream", bufs=3))

    identity = const.tile([P, P], F32)
    make_identity(nc, identity)
    identity_r = const.tile([P, P], F32R, name="identity_r")
    nc.vector.tensor_copy(out=identity_r[:], in_=identity[:])
    gamma_sb = const.tile([P, 1], F32)
    nc.sync.dma_start(gamma_sb[:], g.ap())

    # resident inputs, f32r-rounded by the DMA (matmul/transpose operands only;
    # the exact-x residual is re-streamed in the output phase)
    q_nat = qpool.tile([P, CT, N], F32, name="q_nat", tag="q_nat")
    qd_nat = qpool.tile([P, CT, N], F32R, name="qd_nat", tag="qd_nat")
    # column-block-major load order so the first qT tiles are ready early
    def load_input(dst, dram, cast):
        d = dram.bitcast(F32R) if cast else dram
        cols = [0, 256, 512, 1024, 2048, 3072, 4096]
        for h in range(len(cols) - 1):
            for t in range(CT):
                sl = slice(cols[h], cols[h + 1])
                nc.sync.dma_start(dst[:, t, sl], d[t, :, sl])

    load_input(q_nat, xa, cast=False)
    load_input(qd_nat, xda, cast=True)

    s_sb = qpool.tile([P, CT, FREE], F32, name="s_sb", tag="s_sb")

    with ExitStack() as ectx:
        spsum = ectx.enter_context(tc.tile_pool(name="spsum", bufs=1, space="PSUM"))
        tpsum = ectx.enter_context(tc.tile_pool(name="tpsum", bufs=2, space="PSUM"))

        # S upper-triangle accumulators: one PSUM bank per 128-row tile
        s_ps = [
            spsum.tile([P, FREE], F32, name=f"s{m}", tag=f"s{m}") for m in range(CT)
        ]

        # --- phase E ---
        steps = [(q_nat, k) for k in range(KT)] + [(qd_nat, k) for k in range(KT)]
        n_steps = len(steps)
        pend = {}

        def emit_transpose(i):
            src, k = steps[i]
            is_r = src.dtype == F32R
            tp = tpsum.tile([P, FREE], F32R if is_r else F32, tag="tp", bufs=3)
            for t in range(CT):
                nc.tensor.transpose(
                    tp[:, t * P : (t + 1) * P],
                    src[:, t, k * P : (k + 1) * P],
                    identity_r if is_r else identity,
                )
            qt = stream.tile([P, FREE], F32R, tag="qt")
            if i % 2 == 0:
                nc.vector.tensor_copy(out=qt[:], in_=tp[:])
            else:
                nc.scalar.copy(qt[:], tp[:])
            pend[i] = qt

        def emit_matmuls(i):
            qt = pend.pop(i)
            for m in range(CT):
                lo = m * P
                nc.tensor.matmul(
                    s_ps[m][:, lo:],
                    qt[:, lo : lo + P],
                    qt[:, lo:],
                    start=(i == 0),
                    stop=(i == n_steps - 1),
                )

        # skew by one step: PE transposes of step i+1 overlap the copy of i
        emit_transpose(0)
        for i in range(1, n_steps):
            emit_transpose(i)
            emit_matmuls(i - 1)
        emit_matmuls(n_steps - 1)

        # --- S: upper blocks from PSUM, mirror lower blocks (S symmetric) ---
        for m in range(CT):
            if m % 2 == 0:
                nc.vector.tensor_copy(out=s_sb[:, m, m * P :], in_=s_ps[m][:, m * P :])
            else:
                nc.scalar.copy(s_sb[:, m, m * P :], s_ps[m][:, m * P :])
        for i in range(CT):
            for j in range(i):
                tp = tpsum.tile([P, P], F32, tag="tps", bufs=1)
                nc.tensor.transpose(
                    tp[:], s_sb[:, j, i * P : (i + 1) * P], identity
                )
                nc.vector.tensor_copy(out=s_sb[:, i, j * P : (j + 1) * P], in_=tp[:])

        # --- softmax over rows of -S (stabilizer: rowmax(-S) = -rowmin(S)) ---
        att = qpool.tile([P, CT, FREE], F32, name="att", tag="att")
        rg = []
        for m in range(CT):
            mn = small.tile([P, 1], F32, tag=f"mn{m}")
            nc.vector.tensor_reduce(
                out=mn[:],
                in_=s_sb[:, m, :],
                axis=mybir.AxisListType.X,
                op=mybir.AluOpType.min,
            )
            z = small.tile([P, 1], F32, tag=f"z{m}")
            nc.scalar.activation(
                att[:, m, :],
                s_sb[:, m, :],
                mybir.ActivationFunctionType.Exp,
                bias=mn[:],
                scale=-1.0,
                accum_out=z[:],
            )
            r = small.tile([P, 1], F32, tag=f"r{m}")
            nc.vector.reciprocal(r[:], z[:])
            rgm = small.tile([P, 1], F32, tag=f"rg{m}")
            nc.vector.tensor_mul(out=rgm[:], in0=r[:], in1=gamma_sb[:])
            rg.append(rgm)

        # --- attT (d-major unnormalized att) via 16 PE transposes ---
        attT = qpool.tile([P, CT, FREE], F32R, name="attT", tag="attT")
        for mj in range(CT):
            tp = tpsum.tile([P, FREE], F32, tag="tp", bufs=3)
            for mi in range(CT):
                nc.tensor.transpose(
                    tp[:, mi * P : (mi + 1) * P],
                    att[:, mi, mj * P : (mj + 1) * P],
                    identity,
                )
            nc.vector.tensor_copy(out=attT[:, mj, :], in_=tp[:])

    # --- out = rg * (attT.T @ q) + x, streamed over 8 column tiles ---
    opsum = ctx.enter_context(tc.tile_pool(name="opsum", bufs=4, space="PSUM"))

    def emit_qr(nt):
        nsl = slice(nt * FREE, (nt + 1) * FREE)
        tiles = []
        for kk in range(CT):
            qr = stream.tile([P, FREE], F32R, tag="qr", bufs=8)
            if kk % 2 == 0:
                nc.vector.tensor_copy(out=qr[:], in_=q_nat[:, kk, nsl])
            else:
                nc.scalar.copy(qr[:], q_nat[:, kk, nsl])
            tiles.append(qr)
        return tiles

    q_r = {0: emit_qr(0)}
    for nt in range(NT):
        nsl = slice(nt * FREE, (nt + 1) * FREE)
        tiles = q_r.pop(nt)
        for m in range(CT):
            op_ = opsum.tile([P, FREE], F32, tag="op")
            for kk in range(CT):
                nc.tensor.matmul(
                    op_[:],
                    attT[:, kk, m * P : (m + 1) * P],
                    tiles[kk][:],
                    start=(kk == 0),
                    stop=(kk == CT - 1),
                )
            if m == 0 and nt + 1 < NT:
                q_r[nt + 1] = emit_qr(nt + 1)
            t_sb = ostream.tile([P, FREE], F32, tag="t")
            o_sb = ostream.tile([P, FREE], F32, tag="o")
            if nt == NT - 1:
                # fine-grained tail: quarter-width chunks, engines rotated, so
                # the post-matmul epilogue chain pipelines instead of serializing
                for qtr in range(4):
                    qs = slice(qtr * 128, (qtr + 1) * 128)
                    if qtr % 2 == 0:
                        nc.scalar.mul(t_sb[:, qs], op_[:, qs], rg[m][:])
                        nc.vector.tensor_add(
                            out=o_sb[:, qs], in0=t_sb[:, qs],
                            in1=q_nat[:, m, nsl][:, qs],
                        )
                    else:
                        nc.vector.tensor_scalar_mul(t_sb[:, qs], op_[:, qs], rg[m][:])
                        nc.gpsimd.tensor_add(
                            out=o_sb[:, qs], in0=t_sb[:, qs],
                            in1=q_nat[:, m, nsl][:, qs],
                        )
                    nc.sync.dma_start(oa[m, :, nsl][:, qs], o_sb[:, qs])
            else:
                if m < 2:
                    nc.scalar.mul(t_sb[:], op_[:], rg[m][:])
                else:
                    nc.vector.tensor_scalar_mul(t_sb[:], op_[:], rg[m][:])
                if m % 2 == 1 and nt < NT - 2:
                    nc.gpsimd.tensor_add(out=o_sb[:], in0=t_sb[:], in1=q_nat[:, m, nsl])
                else:
                    nc.vector.tensor_add(out=o_sb[:], in0=t_sb[:], in1=q_nat[:, m, nsl])
                nc.sync.dma_start(oa[m, :, nsl], o_sb[:])


def _build_program():
    global _NC_CACHE
    if _NC_CACHE is not None:
        return _NC_CACHE
    nc = bacc.Bacc("TRN2", target_bir_lowering=False, debug=False)
    x = nc.dram_tensor("x", [C, N], F32, kind="ExternalInput")
    xd = nc.dram_tensor("xd", [C, N], F32, kind="ExternalInput")
    g = nc.dram_tensor("g", [P, 1], F32, kind="ExternalInput")
    o = nc.dram_tensor("o", [C, N], F32, kind="ExternalOutput")

    with tile.TileContext(nc) as tc, ExitStack() as ctx:
        _emit(ctx, tc, nc, x, xd, g, o)
    nc.compile()
    _NC_CACHE = nc
    return nc


def kernel(x_rgb: np.ndarray, x_dep: np.ndarray, gamma: np.ndarray) -> np.ndarray:
    B, Cc, H, W = x_rgb.shape
    assert (B, Cc, H * W) == (8, C, N), (B, Cc, H, W)
    nc = _build_program()
    g128 = np.ascontiguousarray(
        np.broadcast_to(np.float32(gamma).reshape(1, 1), (P, 1)), dtype=np.float32
    )
    in_maps = [
        {
            "x": np.ascontiguousarray(x_rgb[b].reshape(C, N), dtype=np.float32),
            "xd": np.ascontiguousarray(x_dep[b].reshape(C, N), dtype=np.float32),
            "g": g128,
        }
        for b in range(B)
    ]
    res = bass_utils.run_bass_kernel_spmd(nc, in_maps, core_ids=list(range(B)))
    out = np.stack([res.results[b]["o"].reshape(Cc, H, W) for b in range(B)])
    return out.astype(np.float32)


# revision 23
# speedup vs baseline: 1.2494x; 1.0443x over previous
"""RGBD channel-attention (CAM) module on 8 Trainium2 NeuronCores.

Per batch b (one per core, pure data-parallel):
    q  = x_rgb[b].reshape(C, N)          C=512, N=4096
    qd = x_dep[b].reshape(C, N)
    S  = q @ q.T + qd @ qd.T             (512 x 512, symmetric)
    att = softmax(-S, axis=-1)           (max-shift trick cancels in softmax)
    out = gamma * (att @ q) + x_rgb[b]

On-chip schedule per core:
  phase E: build qT / qdT k-tiles via PE transposes, accumulate the upper
           triangle of S = sum_k qT_k^T @ qT_k (both inputs) into 4 PSUM
           banks; matmuls run as float32r (full PE rate).
  S fix:   copy upper blocks to SBUF, mirror the 6 lower blocks via PE
           transposes (S is symmetric).
  softmax: rowwise m=min(S); unnormalized att = exp(-S + m) on ACT with
           accumulated row sum; normalization deferred to the output scale.
  attT:    16 PE transposes (att is needed d-major as matmul lhsT).
  out:     attT.T @ q over d-tiles (f32r); PSUM scaled by gamma/rowsum
           per-partition, residual-added to an exact f32 copy of x, DMA out.
"""

import sys

if "/opt/trn_rl_repo" not in sys.path:
    sys.path.insert(0, "/opt/trn_rl_repo")

from contextlib import ExitStack

import numpy as np

import concourse.bacc as bacc
import concourse.mybir as mybir
import concourse.tile as tile
from concourse import bass_utils
from concourse.masks import make_identity

P = 128          # partitions
C = 512          # channels
N = 4096         # H*W
CT = C // P      # 4 channel tiles
KT = N // P      # 32 contraction tiles per input
FREE = 512       # matmul moving free dim (fp32 max)
NT = N // FREE   # 8 output column tiles
F32 = mybir.dt.float32
F32R = mybir.dt.float32r  # same bits as f32; PE runs full-rate (vs 4 cyc/row for f32)

_NC_CACHE = None


def _emit(ctx, tc, nc, x, xd, g, o):
    xa = x.ap().rearrange("(t p) n -> t p n", p=P)    # [CT, P, N]
    xda = xd.ap().rearrange("(t p) n -> t p n", p=P)
    oa = o.ap().rearrange("(t p) n -> t p n", p=P)

    const = ctx.enter_context(tc.tile_pool(name="const", bufs=1))
    qpool = ctx.enter_context(tc.tile_pool(name="qpool", bufs=1))
    stream = ctx.enter_context(tc.tile_pool(name="stream", bufs=5))
    small = ctx.enter_context(tc.tile_pool(name="small", bufs=1))
    ostream = ctx.enter_context(tc.tile_pool(name="ostream", bufs=3))

    identity = const.tile([P, P], F32)
    make_identity(nc, identity)
    identity_r = const.tile([P, P], F32R, name="identity_r")
    nc.vector.tensor_copy(out=identity_r[:], in_=identity[:])
    gamma_sb = const.tile([P, 1], F32)
    nc.sync.dma_start(gamma_sb[:], g.ap())

    # resident inputs, f32r-rounded by the DMA (matmul/transpose operands only;
    # the exact-x residual is re-streamed in the output phase)
    q_nat = qpool.tile([P, CT, N], F32, name="q_nat", tag="q_nat")
    qd_nat = qpool.tile([P, CT, N], F32R, name="qd_nat", tag="qd_nat")
    # column-block-major load order so the first qT tiles are ready early
    def load_input(dst, dram, cast):
        d = dram.bitcast(F32R) if cast else dram
        cols = [0, 256, 512, 1024, 2048, 3072, 4096]
        for h in range(len(cols) - 1):
            for t in range(CT):
                sl = slice(cols[h], cols[h + 1])
                nc.sync.dma_start(dst[:, t, sl], d[t, :, sl])

    load_input(q_nat, xa, cast=False)
    load_input(qd_nat, xda, cast=True)

    s_sb = qpool.tile([P, CT, FREE], F32, name="s_sb", tag="s_sb")

    with ExitStack() as ectx:
        spsum = ectx.enter_context(tc.tile_pool(name="spsum", bufs=1, space="PSUM"))
        tpsum = ectx.enter_context(tc.tile_pool(name="tpsum", bufs=2, space="PSUM"))

        # S upper-triangle accumulators: one PSUM bank per 128-row tile
        s_ps = [
            spsum.tile([P, FREE], F32, name=f"s{m}", tag=f"s{m}") for m in range(CT)
        ]

        # --- phase E ---
        steps = [(q_nat, k) for k in range(KT)] + [(qd_nat, k) for k in range(KT)]
        n_steps = len(steps)
        pend = {}

        def emit_transpose(i):
            src, k = steps[i]
            is_r = src.dtype == F32R
            tp = tpsum.tile([P, FREE], F32R if is_r else F32, tag="tp", bufs=3)
            for t in range(CT):
                nc.tensor.transpose(
                    tp[:, t * P : (t + 1) * P],
                    src[:, t, k * P : (k + 1) * P],
                    identity_r if is_r else identity,
                )
            qt = stream.tile([P, FREE], F32R, tag="qt")
            if i % 2 == 0:
                nc.vector.tensor_copy(out=qt[:], in_=tp[:])
            else:
                nc.scalar.copy(qt[:], tp[:])
            pend[i] = qt

        def emit_matmuls(i):
            qt = pend.pop(i)
            for m in range(CT):
                lo = m * P
                nc.tensor.matmul(
                    s_ps[m][:, lo:],
                    qt[:, lo : lo + P],
                    qt[:, lo:],
                    start=(i == 0),
                    stop=(i == n_steps - 1),
                )

        # skew by one step: PE transposes of step i+1 overlap the copy of i
        emit_transpose(0)
        for i in range(1, n_steps):
            emit_transpose(i)
            emit_matmuls(i - 1)
        emit_matmuls(n_steps - 1)

        # --- S: upper blocks from PSUM, mirror lower blocks (S symmetric) ---
        for m in range(CT):
            if m % 2 == 0:
                nc.vector.tensor_copy(out=s_sb[:, m, m * P :], in_=s_ps[m][:, m * P :])
            else:
                nc.scalar.copy(s_sb[:, m, m * P :], s_ps[m][:, m * P :])
        for i in range(CT):
            for j in range(i):
                tp = tpsum.tile([P, P], F32, tag="tps", bufs=1)
                nc.tensor.transpose(
                    tp[:], s_sb[:, j, i * P : (i + 1) * P], identity
                )
                nc.vector.tensor_copy(out=s_sb[:, i, j * P : (j + 1) * P], in_=tp[:])

        # --- softmax over rows of -S (stabilizer: rowmax(-S) = -rowmin(S)) ---
        att = qpool.tile([P, CT, FREE], F32, name="att", tag="att")
        rg = []
        for m in range(CT):
            mn = small.tile([P, 1], F32, tag=f"mn{m}")
            nc.vector.tensor_reduce(
                out=mn[:],
                in_=s_sb[:, m, :],
                axis=mybir.AxisListType.X,
                op=mybir.AluOpType.min,
            )
            z = small.tile([P, 1], F32, tag=f"z{m}")
            nc.scalar.activation(
                att[:, m, :],
                s_sb[:, m, :],
                mybir.ActivationFunctionType.Exp,
                bias=mn[:],
                scale=-1.0,
                accum_out=z[:],
            )
            r = small.tile([P, 1], F32, tag=f"r{m}")
            nc.vector.reciprocal(r[:], z[:])
            rgm = small.tile([P, 1], F32, tag=f"rg{m}")
            nc.vector.tensor_mul(out=rgm[:], in0=r[:], in1=gamma_sb[:])
            rg.append(rgm)

        # --- attT (d-major unnormalized att) via 16 PE transposes ---
        attT = qpool.tile([P, CT, FREE], F32R, name="attT", tag="attT")
        for mj in range(CT):
            tp = tpsum.tile([P, FREE], F32, tag="tp", bufs=3)
            for mi in range(CT):
                nc.tensor.transpose(
                    tp[:, mi * P : (mi + 1) * P],
                    att[:, mi, mj * P : (mj + 1) * P],
                    identity,
                )
            nc.vector.tensor_copy(out=attT[:, mj, :], in_=tp[:])

    # --- out = rg * (attT.T @ q) + x, streamed over 8 column tiles ---
    opsum = ctx.enter_context(tc.tile_pool(name="opsum", bufs=4, space="PSUM"))

    def emit_qr(nt):
        nsl = slice(nt * FREE, (nt + 1) * FREE)
        tiles = []
        for kk in range(CT):
            qr = stream.tile([P, FREE], F32R, tag="qr", bufs=8)
            if kk % 2 == 0:
                nc.vector.tensor_copy(out=qr[:], in_=q_nat[:, kk, nsl])
            else:
                nc.scalar.copy(qr[:], q_nat[:, kk, nsl])
            tiles.append(qr)
        return tiles

    q_r = {0: emit_qr(0)}
    for nt in range(NT):
        nsl = slice(nt * FREE, (nt + 1) * FREE)
        tiles = q_r.pop(nt)
        for m in range(CT):
            op_ = opsum.tile([P, FREE], F32, tag="op")
            for kk in range(CT):
                nc.tensor.matmul(
                    op_[:],
                    attT[:, kk, m * P : (m + 1) * P],
                    tiles[kk][:],
                    start=(kk == 0),
                    stop=(kk == CT - 1),
                )
            if m == 0 and nt + 1 < NT:
                q_r[nt + 1] = emit_qr(nt + 1)
            t_sb = ostream.tile([P, FREE], F32, tag="t")
            if m < 2:
                nc.scalar.mul(t_sb[:], op_[:], rg[m][:])
            else:
                nc.vector.tensor_scalar_mul(t_sb[:], op_[:], rg[m][:])
            o_sb = ostream.tile([P, FREE], F32, tag="o")
            if m % 2 == 1 and nt < NT - 2:
                nc.gpsimd.tensor_add(out=o_sb[:], in0=t_sb[:], in1=q_nat[:, m, nsl])
            else:
                nc.vector.tensor_add(out=o_sb[:], in0=t_sb[:], in1=q_nat[:, m, nsl])
            nc.sync.dma_start(oa[m, :, nsl], o_sb[:])


def _build_program():
    global _NC_CACHE
    if _NC_CACHE is not None:
        return _NC_CACHE
    nc = bacc.Bacc("TRN2", target_bir_lowering=False, debug=False)
    x = nc.dram_tensor("x", [C, N], F32, kind="ExternalInput")
    xd = nc.dram_tensor("xd", [C, N], F32, kind="ExternalInput")
    g = nc.dram_tensor("g", [P, 1], F32, kind="ExternalInput")
    o = nc.dram_tensor("o", [C, N], F32, kind="ExternalOutput")

    with tile.TileContext(nc) as tc, ExitStack() as ctx:
        _emit(ctx, tc, nc, x, xd, g, o)
    nc.compile()
    _NC_CACHE = nc
    return nc


def kernel(x_rgb: np.ndarray, x_dep: np.ndarray, gamma: np.ndarray) -> np.ndarray:
    B, Cc, H, W = x_rgb.shape
    assert (B, Cc, H * W) == (8, C, N), (B, Cc, H, W)
    nc = _build_program()
    g128 = np.ascontiguousarray(
        np.broadcast_to(np.float32(gamma).reshape(1, 1), (P, 1)), dtype=np.float32
    )
    in_maps = [
        {
            "x": np.ascontiguousarray(x_rgb[b].reshape(C, N), dtype=np.float32),
            "xd": np.ascontiguousarray(x_dep[b].reshape(C, N), dtype=np.float32),
            "g": g128,
        }
        for b in range(B)
    ]
    res = bass_utils.run_bass_kernel_spmd(nc, in_maps, core_ids=list(range(B)))
    out = np.stack([res.results[b]["o"].reshape(Cc, H, W) for b in range(B)])
    return out.astype(np.float32)


# revision 24
# speedup vs baseline: 1.2722x; 1.0183x over previous
"""RGBD channel-attention (CAM) module on 8 Trainium2 NeuronCores.

Per batch b (one per core, pure data-parallel):
    q  = x_rgb[b].reshape(C, N)          C=512, N=4096
    qd = x_dep[b].reshape(C, N)
    S  = q @ q.T + qd @ qd.T             (512 x 512, symmetric)
    att = softmax(-S, axis=-1)           (max-shift trick cancels in softmax)
    out = gamma * (att @ q) + x_rgb[b]

On-chip schedule per core:
  phase E: build qT / qdT k-tiles via PE transposes, accumulate the upper
           triangle of S = sum_k qT_k^T @ qT_k (both inputs) into 4 PSUM
           banks; matmuls run as float32r (full PE rate).
  S fix:   copy upper blocks to SBUF, mirror the 6 lower blocks via PE
           transposes (S is symmetric).
  softmax: rowwise m=min(S); unnormalized att = exp(-S + m) on ACT with
           accumulated row sum; normalization deferred to the output scale.
  attT:    16 PE transposes (att is needed d-major as matmul lhsT).
  out:     attT.T @ q over d-tiles (f32r); PSUM scaled by gamma/rowsum
           per-partition, residual-added to an exact f32 copy of x, DMA out.
"""

import sys

if "/opt/trn_rl_repo" not in sys.path:
    sys.path.insert(0, "/opt/trn_rl_repo")

from contextlib import ExitStack

import numpy as np

import concourse.bacc as bacc
import concourse.mybir as mybir
import concourse.tile as tile
from concourse import bass_utils
from concourse.masks import make_identity

P = 128          # partitions
C = 512          # channels
N = 4096         # H*W
CT = C // P      # 4 channel tiles
KT = N // P      # 32 contraction tiles per input
FREE = 512       # matmul moving free dim (fp32 max)
NT = N // FREE   # 8 output column tiles
F32 = mybir.dt.float32
F32R = mybir.dt.float32r  # same bits as f32; PE runs full-rate (vs 4 cyc/row for f32)

_NC_CACHE = None


def _emit(ctx, tc, nc, x, xd, g, o):
    xa = x.ap().rearrange("(t p) n -> t p n", p=P)    # [CT, P, N]
    xda = xd.ap().rearrange("(t p) n -> t p n", p=P)
    oa = o.ap().rearrange("(t p) n -> t p n", p=P)

    const = ctx.enter_context(tc.tile_pool(name="const", bufs=1))
    qpool = ctx.enter_context(tc.tile_pool(name="qpool", bufs=1))
    stream = ctx.enter_context(tc.tile_pool(name="stream", bufs=5))
    small = ctx.enter_context(tc.tile_pool(name="small", bufs=1))
    ostream = ctx.enter_context(tc.tile_pool(name="ostream", bufs=3))

    identity = const.tile([P, P], F32)
    make_identity(nc, identity)
    identity_r = const.tile([P, P], F32R, name="identity_r")
    nc.vector.tensor_copy(out=identity_r[:], in_=identity[:])
    gamma_sb = const.tile([P, 1], F32)
    nc.sync.dma_start(gamma_sb[:], g.ap())

    # resident inputs, f32r-rounded by the DMA (matmul/transpose operands only;
    # the exact-x residual is re-streamed in the output phase)
    q_nat = qpool.tile([P, CT, N], F32, name="q_nat", tag="q_nat")
    qd_nat = qpool.tile([P, CT, N], F32R, name="qd_nat", tag="qd_nat")
    # column-block-major load order so the first qT tiles are ready early
    def load_input(dst, dram, cast):
        d = dram.bitcast(F32R) if cast else dram
        cols = [0, 256, 512, 1024, 2048, 3072, 4096]
        for h in range(len(cols) - 1):
            for t in range(CT):
                sl = slice(cols[h], cols[h + 1])
                nc.sync.dma_start(dst[:, t, sl], d[t, :, sl])

    load_input(q_nat, xa, cast=False)
    load_input(qd_nat, xda, cast=True)

    s_sb = qpool.tile([P, CT, FREE], F32, name="s_sb", tag="s_sb")

    with ExitStack() as ectx:
        spsum = ectx.enter_context(tc.tile_pool(name="spsum", bufs=1, space="PSUM"))
        tpsum = ectx.enter_context(tc.tile_pool(name="tpsum", bufs=2, space="PSUM"))

        # S upper-triangle accumulators: one PSUM bank per 128-row tile
        s_ps = [
            spsum.tile([P, FREE], F32, name=f"s{m}", tag=f"s{m}") for m in range(CT)
        ]

        # --- phase E ---
        steps = [(q_nat, k) for k in range(KT)] + [(qd_nat, k) for k in range(KT)]
        n_steps = len(steps)
        pend = {}

        def emit_transpose(i):
            src, k = steps[i]
            is_r = src.dtype == F32R
            tp = tpsum.tile([P, FREE], F32R if is_r else F32, tag="tp", bufs=3)
            for t in range(CT):
                nc.tensor.transpose(
                    tp[:, t * P : (t + 1) * P],
                    src[:, t, k * P : (k + 1) * P],
                    identity_r if is_r else identity,
                )
            qt = stream.tile([P, FREE], F32R, tag="qt")
            if i % 2 == 0:
                nc.vector.tensor_copy(out=qt[:], in_=tp[:])
            else:
                nc.scalar.copy(qt[:], tp[:])
            pend[i] = qt

        def emit_matmuls(i):
            qt = pend.pop(i)
            for m in range(CT):
                lo = m * P
                nc.tensor.matmul(
                    s_ps[m][:, lo:],
                    qt[:, lo : lo + P],
                    qt[:, lo:],
                    start=(i == 0),
                    stop=(i == n_steps - 1),
                )

        # skew by one step: PE transposes of step i+1 overlap the copy of i
        emit_transpose(0)
        for i in range(1, n_steps):
            emit_transpose(i)
            emit_matmuls(i - 1)
        emit_matmuls(n_steps - 1)

        # --- S: upper blocks from PSUM, mirror lower blocks (S symmetric) ---
        for m in range(CT):
            if m % 2 == 0:
                nc.vector.tensor_copy(out=s_sb[:, m, m * P :], in_=s_ps[m][:, m * P :])
            else:
                nc.scalar.copy(s_sb[:, m, m * P :], s_ps[m][:, m * P :])
        for i in range(CT):
            for j in range(i):
                tp = tpsum.tile([P, P], F32, tag="tps", bufs=1)
                nc.tensor.transpose(
                    tp[:], s_sb[:, j, i * P : (i + 1) * P], identity
                )
                nc.vector.tensor_copy(out=s_sb[:, i, j * P : (j + 1) * P], in_=tp[:])

        # --- softmax over rows of -S (stabilizer: rowmax(-S) = -rowmin(S)) ---
        att = qpool.tile([P, CT, FREE], F32, name="att", tag="att")
        rg = []
        for m in range(CT):
            mn = small.tile([P, 1], F32, tag=f"mn{m}")
            nc.vector.tensor_reduce(
                out=mn[:],
                in_=s_sb[:, m, :],
                axis=mybir.AxisListType.X,
                op=mybir.AluOpType.min,
            )
            z = small.tile([P, 1], F32, tag=f"z{m}")
            nc.scalar.activation(
                att[:, m, :],
                s_sb[:, m, :],
                mybir.ActivationFunctionType.Exp,
                bias=mn[:],
                scale=-1.0,
                accum_out=z[:],
            )
            r = small.tile([P, 1], F32, tag=f"r{m}")
            nc.vector.reciprocal(r[:], z[:])
            rgm = small.tile([P, 1], F32, tag=f"rg{m}")
            nc.vector.tensor_mul(out=rgm[:], in0=r[:], in1=gamma_sb[:])
            rg.append(rgm)

        # --- attT (d-major unnormalized att) via 16 PE transposes ---
        attT = qpool.tile([P, CT, FREE], F32R, name="attT", tag="attT")
        for mj in range(CT):
            tp = tpsum.tile([P, FREE], F32, tag="tp", bufs=3)
            for mi in range(CT):
                nc.tensor.transpose(
                    tp[:, mi * P : (mi + 1) * P],
                    att[:, mi, mj * P : (mj + 1) * P],
                    identity,
                )
            nc.vector.tensor_copy(out=attT[:, mj, :], in_=tp[:])

    # --- out = rg * (attT.T @ q) + x, streamed over 8 column tiles ---
    opsum = ctx.enter_context(tc.tile_pool(name="opsum", bufs=6, space="PSUM"))

    def emit_qr(nt):
        nsl = slice(nt * FREE, (nt + 1) * FREE)
        tiles = []
        for kk in range(CT):
            qr = stream.tile([P, FREE], F32R, tag="qr", bufs=8)
            if kk % 2 == 0:
                nc.vector.tensor_copy(out=qr[:], in_=q_nat[:, kk, nsl])
            else:
                nc.scalar.copy(qr[:], q_nat[:, kk, nsl])
            tiles.append(qr)
        return tiles

    q_r = {0: emit_qr(0)}
    for nt in range(NT):
        nsl = slice(nt * FREE, (nt + 1) * FREE)
        tiles = q_r.pop(nt)
        for m in range(CT):
            op_ = opsum.tile([P, FREE], F32, tag="op")
            for kk in range(CT):
                nc.tensor.matmul(
                    op_[:],
                    attT[:, kk, m * P : (m + 1) * P],
                    tiles[kk][:],
                    start=(kk == 0),
                    stop=(kk == CT - 1),
                )
            if m == 0 and nt + 1 < NT:
                q_r[nt + 1] = emit_qr(nt + 1)
            t_sb = ostream.tile([P, FREE], F32, tag="t")
            late = nt >= NT - 2
            if late or m < 2:
                nc.scalar.mul(t_sb[:], op_[:], rg[m][:])
            else:
                nc.vector.tensor_scalar_mul(t_sb[:], op_[:], rg[m][:])
            o_sb = ostream.tile([P, FREE], F32, tag="o")
            if m % 2 == 1 and not late:
                nc.gpsimd.tensor_add(out=o_sb[:], in0=t_sb[:], in1=q_nat[:, m, nsl])
            else:
                nc.vector.tensor_add(out=o_sb[:], in0=t_sb[:], in1=q_nat[:, m, nsl])
            nc.sync.dma_start(oa[m, :, nsl], o_sb[:])


def _build_program():
    global _NC_CACHE
    if _NC_CACHE is not None:
        return _NC_CACHE
    nc = bacc.Bacc("TRN2", target_bir_lowering=False, debug=False)
    x = nc.dram_tensor("x", [C, N], F32, kind="ExternalInput")
    xd = nc.dram_tensor("xd", [C, N], F32, kind="ExternalInput")
    g = nc.dram_tensor("g", [P, 1], F32, kind="ExternalInput")
    o = nc.dram_tensor("o", [C, N], F32, kind="ExternalOutput")

    with tile.TileContext(nc) as tc, ExitStack() as ctx:
        _emit(ctx, tc, nc, x, xd, g, o)
    nc.compile()
    _NC_CACHE = nc
    return nc


def kernel(x_rgb: np.ndarray, x_dep: np.ndarray, gamma: np.ndarray) -> np.ndarray:
    B, Cc, H, W = x_rgb.shape
    assert (B, Cc, H * W) == (8, C, N), (B, Cc, H, W)
    nc = _build_program()
    g128 = np.ascontiguousarray(
        np.broadcast_to(np.float32(gamma).reshape(1, 1), (P, 1)), dtype=np.float32
    )
    in_maps = [
        {
            "x": np.ascontiguousarray(x_rgb[b].reshape(C, N), dtype=np.float32),
            "xd": np.ascontiguousarray(x_dep[b].reshape(C, N), dtype=np.float32),
            "g": g128,
        }
        for b in range(B)
    ]
    res = bass_utils.run_bass_kernel_spmd(nc, in_maps, core_ids=list(range(B)))
    out = np.stack([res.results[b]["o"].reshape(Cc, H, W) for b in range(B)])
    return out.astype(np.float32)


# revision 26
# speedup vs baseline: 1.2987x; 1.0208x over previous
"""RGBD channel-attention (CAM) module on 8 Trainium2 NeuronCores.

Per batch b (one per core, pure data-parallel):
    q  = x_rgb[b].reshape(C, N)          C=512, N=4096
    qd = x_dep[b].reshape(C, N)
    S  = q @ q.T + qd @ qd.T             (512 x 512, symmetric)
    att = softmax(-S, axis=-1)           (max-shift trick cancels in softmax)
    out = gamma * (att @ q) + x_rgb[b]

On-chip schedule per core:
  phase E: build qT / qdT k-tiles via PE transposes, accumulate the upper
           triangle of S = sum_k qT_k^T @ qT_k (both inputs) into 4 PSUM
           banks; matmuls run as float32r (full PE rate).
  S fix:   copy upper blocks to SBUF, mirror the 6 lower blocks via PE
           transposes (S is symmetric).
  softmax: rowwise m=min(S); unnormalized att = exp(-S + m) on ACT with
           accumulated row sum; normalization deferred to the output scale.
  attT:    16 PE transposes (att is needed d-major as matmul lhsT).
  out:     attT.T @ q over d-tiles (f32r); PSUM scaled by gamma/rowsum
           per-partition, residual-added to an exact f32 copy of x, DMA out.
"""

import sys

if "/opt/trn_rl_repo" not in sys.path:
    sys.path.insert(0, "/opt/trn_rl_repo")

from contextlib import ExitStack

import numpy as np

import concourse.bacc as bacc
import concourse.mybir as mybir
import concourse.tile as tile
from concourse import bass_utils
from concourse.masks import make_identity

P = 128          # partitions
C = 512          # channels
N = 4096         # H*W
CT = C // P      # 4 channel tiles
KT = N // P      # 32 contraction tiles per input
FREE = 512       # matmul moving free dim (fp32 max)
NT = N // FREE   # 8 output column tiles
F32 = mybir.dt.float32
F32R = mybir.dt.float32r  # same bits as f32; PE runs full-rate (vs 4 cyc/row for f32)

_NC_CACHE = None


def _emit(ctx, tc, nc, x, xd, g, o):
    xa = x.ap().rearrange("(t p) n -> t p n", p=P)    # [CT, P, N]
    xda = xd.ap().rearrange("(t p) n -> t p n", p=P)
    oa = o.ap().rearrange("(t p) n -> t p n", p=P)

    const = ctx.enter_context(tc.tile_pool(name="const", bufs=1))
    qpool = ctx.enter_context(tc.tile_pool(name="qpool", bufs=1))
    stream = ctx.enter_context(tc.tile_pool(name="stream", bufs=5))
    small = ctx.enter_context(tc.tile_pool(name="small", bufs=1))
    ostream = ctx.enter_context(tc.tile_pool(name="ostream", bufs=3))

    identity = const.tile([P, P], F32)
    make_identity(nc, identity)
    identity_r = const.tile([P, P], F32R, name="identity_r")
    nc.vector.tensor_copy(out=identity_r[:], in_=identity[:])
    gamma_sb = const.tile([P, 1], F32)
    nc.sync.dma_start(gamma_sb[:], g.ap())

    # resident inputs, f32r-rounded by the DMA (matmul/transpose operands only;
    # the exact-x residual is re-streamed in the output phase)
    q_nat = qpool.tile([P, CT, N], F32, name="q_nat", tag="q_nat")
    qd_nat = qpool.tile([P, CT, N], F32R, name="qd_nat", tag="qd_nat")
    # column-block-major load order so the first qT tiles are ready early
    def load_input(dst, dram, cast):
        d = dram.bitcast(F32R) if cast else dram
        cols = [0, 256, 512, 1024, 2048, 3072, 4096]
        for h in range(len(cols) - 1):
            for t in range(CT):
                sl = slice(cols[h], cols[h + 1])
                nc.sync.dma_start(dst[:, t, sl], d[t, :, sl])

    load_input(q_nat, xa, cast=False)
    load_input(qd_nat, xda, cast=True)

    s_sb = qpool.tile([P, CT, FREE], F32, name="s_sb", tag="s_sb")

    with ExitStack() as ectx:
        spsum = ectx.enter_context(tc.tile_pool(name="spsum", bufs=1, space="PSUM"))
        tpsum = ectx.enter_context(tc.tile_pool(name="tpsum", bufs=2, space="PSUM"))

        # S upper-triangle accumulators: one PSUM bank per 128-row tile
        s_ps = [
            spsum.tile([P, FREE], F32, name=f"s{m}", tag=f"s{m}") for m in range(CT)
        ]

        # --- phase E ---
        steps = [(q_nat, k) for k in range(KT)] + [(qd_nat, k) for k in range(KT)]
        n_steps = len(steps)
        pend = {}

        def emit_transpose(i):
            src, k = steps[i]
            is_r = src.dtype == F32R
            tp = tpsum.tile([P, FREE], F32R if is_r else F32, tag="tp", bufs=3)
            for t in range(CT):
                nc.tensor.transpose(
                    tp[:, t * P : (t + 1) * P],
                    src[:, t, k * P : (k + 1) * P],
                    identity_r if is_r else identity,
                )
            qt = stream.tile([P, FREE], F32R, tag="qt")
            if i % 2 == 0:
                nc.vector.tensor_copy(out=qt[:], in_=tp[:])
            else:
                nc.scalar.copy(qt[:], tp[:])
            pend[i] = qt

        def emit_matmuls(i):
            qt = pend.pop(i)
            for m in range(CT):
                lo = m * P
                nc.tensor.matmul(
                    s_ps[m][:, lo:],
                    qt[:, lo : lo + P],
                    qt[:, lo:],
                    start=(i == 0),
                    stop=(i == n_steps - 1),
                )

        # skew by one step: PE transposes of step i+1 overlap the copy of i
        emit_transpose(0)
        for i in range(1, n_steps):
            emit_transpose(i)
            emit_matmuls(i - 1)
        emit_matmuls(n_steps - 1)

        # --- S: upper blocks from PSUM, mirror lower blocks (S symmetric) ---
        for m in range(CT):
            if m % 2 == 0:
                nc.vector.tensor_copy(out=s_sb[:, m, m * P :], in_=s_ps[m][:, m * P :])
            else:
                nc.scalar.copy(s_sb[:, m, m * P :], s_ps[m][:, m * P :])
        for i in range(CT):
            for j in range(i):
                tp = tpsum.tile([P, P], F32, tag="tps", bufs=1)
                nc.tensor.transpose(
                    tp[:], s_sb[:, j, i * P : (i + 1) * P], identity
                )
                nc.vector.tensor_copy(out=s_sb[:, i, j * P : (j + 1) * P], in_=tp[:])

        # --- softmax over rows of -S (stabilizer: rowmax(-S) = -rowmin(S)) ---
        att = qpool.tile([P, CT, FREE], F32, name="att", tag="att")
        rg = []
        for m in range(CT):
            mn = small.tile([P, 1], F32, tag=f"mn{m}")
            nc.vector.tensor_reduce(
                out=mn[:],
                in_=s_sb[:, m, :],
                axis=mybir.AxisListType.X,
                op=mybir.AluOpType.min,
            )
            z = small.tile([P, 1], F32, tag=f"z{m}")
            nc.scalar.activation(
                att[:, m, :],
                s_sb[:, m, :],
                mybir.ActivationFunctionType.Exp,
                bias=mn[:],
                scale=-1.0,
                accum_out=z[:],
            )
            r = small.tile([P, 1], F32, tag=f"r{m}")
            nc.vector.reciprocal(r[:], z[:])
            rgm = small.tile([P, 1], F32, tag=f"rg{m}")
            nc.vector.tensor_mul(out=rgm[:], in0=r[:], in1=gamma_sb[:])
            rg.append(rgm)

        # --- attT (d-major unnormalized att) via 16 PE transposes ---
        attT = qpool.tile([P, CT, FREE], F32R, name="attT", tag="attT")
        for mj in range(CT):
            tp = tpsum.tile([P, FREE], F32, tag="tp", bufs=3)
            for mi in range(CT):
                nc.tensor.transpose(
                    tp[:, mi * P : (mi + 1) * P],
                    att[:, mi, mj * P : (mj + 1) * P],
                    identity,
                )
            nc.vector.tensor_copy(out=attT[:, mj, :], in_=tp[:])

    # --- out = rg * (attT.T @ q) + x, streamed over 8 column tiles ---
    opsum = ctx.enter_context(tc.tile_pool(name="opsum", bufs=6, space="PSUM"))

    def emit_qr(nt):
        nsl = slice(nt * FREE, (nt + 1) * FREE)
        tiles = []
        for kk in range(CT):
            qr = stream.tile([P, FREE], F32R, tag="qr", bufs=8)
            if kk % 2 == 0:
                nc.vector.tensor_copy(out=qr[:], in_=q_nat[:, kk, nsl])
            else:
                nc.scalar.copy(qr[:], q_nat[:, kk, nsl])
            tiles.append(qr)
        return tiles

    q_r = {0: emit_qr(0)}
    for nt in range(NT):
        nsl = slice(nt * FREE, (nt + 1) * FREE)
        tiles = q_r.pop(nt)
        for m in range(CT):
            op_ = opsum.tile([P, FREE], F32, tag="op")
            for kk in range(CT):
                nc.tensor.matmul(
                    op_[:],
                    attT[:, kk, m * P : (m + 1) * P],
                    tiles[kk][:],
                    start=(kk == 0),
                    stop=(kk == CT - 1),
                )
            if m == 0 and nt + 1 < NT:
                q_r[nt + 1] = emit_qr(nt + 1)
            t_sb = ostream.tile([P, FREE], F32, tag="t")
            late = nt >= NT - 2
            if late or m < 2:
                nc.scalar.mul(t_sb[:], op_[:], rg[m][:])
            else:
                nc.vector.tensor_scalar_mul(t_sb[:], op_[:], rg[m][:])
            o_sb = ostream.tile([P, FREE], F32, tag="o")
            if m % 2 == 1 and not late:
                nc.gpsimd.tensor_add(out=o_sb[:], in0=t_sb[:], in1=q_nat[:, m, nsl])
            else:
                nc.vector.tensor_add(out=o_sb[:], in0=t_sb[:], in1=q_nat[:, m, nsl])
            nc.sync.dma_start(oa[m, :, nsl], o_sb[:])


def _build_program():
    global _NC_CACHE
    if _NC_CACHE is not None:
        return _NC_CACHE
    nc = bacc.Bacc("TRN2", target_bir_lowering=False, debug=False)
    x = nc.dram_tensor("x", [C, N], F32, kind="ExternalInput")
    xd = nc.dram_tensor("xd", [C, N], F32, kind="ExternalInput")
    g = nc.dram_tensor("g", [P, 1], F32, kind="ExternalInput")
    o = nc.dram_tensor("o", [C, N], F32, kind="ExternalOutput")

    with tile.TileContext(nc) as tc, ExitStack() as ctx:
        _emit(ctx, tc, nc, x, xd, g, o)
    nc.compile()
    _NC_CACHE = nc
    return nc


def kernel(x_rgb: np.ndarray, x_dep: np.ndarray, gamma: np.ndarray) -> np.ndarray:
    B, Cc, H, W = x_rgb.shape
    assert (B, Cc, H * W) == (8, C, N), (B, Cc, H, W)
    nc = _build_program()
    g128 = np.ascontiguousarray(
        np.broadcast_to(np.float32(gamma).reshape(1, 1), (P, 1)), dtype=np.float32
    )
    in_maps = [
        {
            "x": np.ascontiguousarray(x_rgb[b].reshape(C, N), dtype=np.float32),
            "xd": np.ascontiguousarray(x_dep[b].reshape(C, N), dtype=np.float32),
            "g": g128,
        }
        for b in range(B)
    ]
    res = bass_utils.run_bass_kernel_spmd(nc, in_maps, core_ids=list(range(B)))
    out = np.stack([res.results[b]["o"].reshape(Cc, H, W) for b in range(B)])
    return out.astype(np.float32)
